# revision 2
# baseline (speedup 1.0000x reference)
"""C-index kernel v2 for Trainium2, 8 NeuronCores — bucket-matmul scheme.

Math (bf16-rounded world; error vs fp32 reference ~1e-3 << 2e-2 gate)
----
y, yh are rounded to fp16 on host.  Over the rectangle (i in [N]) x
(j in events E):
    S1 = sum [y_i>=y_j][yh_i>=yh_j],   S2 = sum [y_i>=y_j]
    c = S1 - ns, t = S2 - ns, out = fp32(c)/fp32(t)
yh is rank-bucketed into K=128 equal buckets (128 samples each), q(.):
    [yh_i>=yh_j] = [q_i>q_j] + [q_i==q_j][yh_i>=yh_j]
so  S1 = S1_cross + S1_same:
    S1_cross = sum_j sum_{k>q_j} cnt[k, j],  cnt[k,j] = #{i: q_i=k, y_i>=y_j}
    S1_same  = exact pairwise count within each yh-bucket (1/128 of work)
and S2 = sum_{k,j} cnt[k,j] (every i is in exactly one bucket).

Device (i sharded 8 ways, 2048 rows = 16 tiles of 128 per core; j = all
packed events JE on the free axis):
  - compare pass: A[i,j] = [y_j <= y_i] as fp8, one op per tile on
    DVE/GpSimd (is_le 0/1) or ScalarE (Sign +-1, absorbed by -0.5
    one-hot weights + affine host fix).
  - counting: TensorE DoubleRow fp8 matmuls contract A against one-hot
    yh-bucket weights U -> cnt[k, j-chunk] in PSUM, DMA'd to DRAM.
  - same-bucket pass: 16 buckets/core packed [128 members x W events],
    3 fused fp16 DVE ops -> S1_same partials.
Host: pack inputs / one-hots, then exact fp64 algebra over cnt
(suffix-cumsum + gather) and the same-bucket accumulators.
"""

import math
import os
import sys

import numpy as np
import ml_dtypes

for _p in ("/opt/trn_rl_repo", "/root/.axon_site", "/root/.axon_site/_ro/trn_rl_repo"):
    if os.path.isdir(_p) and _p not in sys.path:
        sys.path.append(_p)

import concourse.bacc as bacc
import concourse.mybir as mybir
from concourse import bass_utils
from concourse import tile

N = 16384
P = 128
NCORES = 8
TPC = N // NCORES          # 2048 i per core
NT = TPC // P              # 16 i-tiles per core
K = 128                    # yh buckets (N/K = 128 exactly)
CH = 2048                  # j-chunk width (PSUM: [128, 2048] f32 = 4 banks)
PAD = ml_dtypes.bfloat16(1e30)  # +inf-ish pad, bf16-exact

FP32 = mybir.dt.float32
FP16 = mybir.dt.float16
BF16 = mybir.dt.bfloat16
FP8 = mybir.dt.float8e4
Alu = mybir.AluOpType
ActF = mybir.ActivationFunctionType
DR = mybir.MatmulPerfMode.DoubleRow

# per-(tile, j-half) engine plan: 'v' = DVE is_le, 's' = ScalarE Sign,
# 'g' = GpSimd is_le.  Mixing v/g within a tile is fine (same 0/1
# semantics and +1 one-hot weights); sign tiles are whole tiles.
# PAIR_ORDER puts fast pairs first in each PSUM accumulation chain.
# NOTE: GpSimd ('g') is banned — concurrent Pool+DVE ops slow each
# other ~25x on TRN2 (measured), so only DVE + ScalarE compare.
HALF_PLAN = [
    ('v', 'v'), ('v', 'v'), ('v', 'v'), ('v', 'v'),
    ('v', 'v'), ('v', 'v'), ('v', 'v'), ('v', 'v'),
    ('v', 'v'), ('v', 'v'), ('s', 's'), ('s', 's'),
    ('s', 's'), ('s', 's'), ('s', 's'), ('s', 's'),
]
SIGN_TILES = frozenset(t for t, e in enumerate(HALF_PLAN) if e == ('s', 's'))
PAIR_ORDER = [0, 1, 2, 3, 4, 5, 6, 7]   # vv x4, v/g+gg x2, ss x2


def build_bass_full(je, sbw):
    """je: padded event count (mult of 128); sbw: same-bucket width."""
    n_ch = int(math.ceil(je / CH))
    nc = bacc.Bacc(debug=False, num_devices=NCORES)

    yj = nc.dram_tensor("yj", [1, je], BF16, kind="ExternalInput")
    y_is = nc.dram_tensor("y_is", [P, NT], FP32, kind="ExternalInput")
    u_w = nc.dram_tensor("u_w", [P, NT * K], FP8, kind="ExternalInput")
    sb_yj = nc.dram_tensor("sb_yj", [P, sbw], BF16, kind="ExternalInput")
    sb_yi = nc.dram_tensor("sb_yi", [P, sbw], BF16, kind="ExternalInput")
    sb_hj = nc.dram_tensor("sb_hj", [P, sbw], BF16, kind="ExternalInput")
    sb_hi = nc.dram_tensor("sb_hi", [P, sbw], BF16, kind="ExternalInput")
    o_cnt = nc.dram_tensor("o_cnt", [P, je], FP16, kind="ExternalOutput")
    o_sbacc = nc.dram_tensor("o_sbacc", [P, 1], FP32, kind="ExternalOutput")

    with tile.TileContext(nc) as tc:
        with (
            tc.tile_pool(name="const", bufs=1) as cpool,
            tc.tile_pool(name="a", bufs=1) as apool,
            tc.tile_pool(name="scrap", bufs=1) as spool,
            tc.tile_pool(name="psum", bufs=2, space="PSUM") as ppool,
        ):
            yjb = cpool.tile([P, je], BF16, name="yjb")
            nc.sync.dma_start(out=yjb[:, :], in_=yj[0:1, :].to_broadcast((P, je)))
            yis = cpool.tile([P, NT], FP32, name="yis")
            nc.sync.dma_start(out=yis[:, :], in_=y_is[:, :])
            uw = cpool.tile([P, NT, K], FP8, name="uw")
            nc.sync.dma_start(out=uw[:, :, :], in_=u_w[:, :])

            # A pairs: [P, 2, je] fp8, pair p = tiles (2p, 2p+1)
            a_pairs = [apool.tile([P, 2, je], FP8, name=f"apair{p}")
                       for p in range(NT // 2)]

            # ---- same-bucket pass first (DVE, fp16, fused, small) ----
            sbyj = cpool.tile([P, sbw], BF16, name="sbyj")
            nc.sync.dma_start(out=sbyj[:, :], in_=sb_yj[:, :])
            sbyi = cpool.tile([P, sbw], BF16, name="sbyi")
            nc.sync.dma_start(out=sbyi[:, :], in_=sb_yi[:, :])
            sbhj = cpool.tile([P, sbw], BF16, name="sbhj")
            nc.sync.dma_start(out=sbhj[:, :], in_=sb_hj[:, :])
            sbhi = cpool.tile([P, sbw], BF16, name="sbhi")
            nc.sync.dma_start(out=sbhi[:, :], in_=sb_hi[:, :])
            sb_a = spool.tile([P, sbw], FP8, name="sb_a")
            nc.vector.tensor_tensor(
                out=sb_a[:, :], in0=sbyj[:, :], in1=sbyi[:, :], op=Alu.is_le)
            sb_b = spool.tile([P, sbw], FP8, name="sb_b")
            nc.vector.tensor_tensor(
                out=sb_b[:, :], in0=sbhj[:, :], in1=sbhi[:, :], op=Alu.is_le)
            sbacc = cpool.tile([P, 1], FP32, name="sbacc")
            sb_p = spool.tile([P, sbw], FP8, name="sb_p")
            nc.vector.scalar_tensor_tensor(
                out=sb_p[:, :], in0=sb_a[:, :], scalar=1.0, in1=sb_b[:, :],
                op0=Alu.mult, op1=Alu.mult, accum_out=sbacc[:, :])
            nc.sync.dma_start(out=o_sbacc[:, :], in_=sbacc[:, :])

            # ---- compare pass: one op per (tile, j-half) ----
            jsplit = (je // 256) * 128   # rough half, 128-aligned
            negs = {}
            for t in range(NT):
                if 's' in HALF_PLAN[t]:
                    neg = cpool.tile([P, 1], FP32, name=f"neg{t}")
                    nc.vector.tensor_scalar_mul(
                        neg[:, :], yis[:, t:t + 1], -1.0)
                    negs[t] = neg
            for h in range(2):
                j0, j1 = (0, jsplit) if h == 0 else (jsplit, je)
                for t in range(NT):
                    dst = a_pairs[t // 2][:, t % 2, j0:j1]
                    eng = HALF_PLAN[t][h]
                    if eng == 's':
                        nc.scalar.activation(
                            out=dst, in_=yjb[:, j0:j1], func=ActF.Sign,
                            bias=negs[t][:, 0:1], scale=1.0)
                    else:
                        e = nc.vector if eng == 'v' else nc.gpsimd
                        e.tensor_scalar(
                            out=dst, in0=yjb[:, j0:j1],
                            scalar1=yis[:, t:t + 1],
                            scalar2=None, op0=Alu.is_le)

            # ---- counting matmuls per j-chunk; cnt -> fp16 -> DRAM ----
            # (DMA cannot read PSUM; ScalarE copies with fp16 downcast,
            # exact for integer counts <= 2048)
            for ch in range(n_ch):
                c0 = ch * CH
                cw = min(CH, je - c0)
                cnt = ppool.tile([P, CH], FP32, name=f"cnt{ch}", tag="cnt")
                # matmul out is capped at one PSUM bank (512 f32)
                for s0 in range(0, cw, 512):
                    sw = min(512, cw - s0)
                    for pi, p in enumerate(PAIR_ORDER):
                        nc.tensor.matmul(
                            cnt[:, s0:s0 + sw],
                            uw[:, 2 * p:2 * p + 2, :],
                            a_pairs[p][:, :, c0 + s0:c0 + s0 + sw],
                            start=(pi == 0), stop=(pi == NT // 2 - 1),
                            perf_mode=DR)
                ccopy = spool.tile([P, CH], FP16, name=f"ccopy{ch % 2}",
                                   tag=f"ccopy{ch % 2}")
                nc.scalar.activation(
                    out=ccopy[:, 0:cw], in_=cnt[:, 0:cw], func=ActF.Copy,
                    bias=0.0, scale=1.0)
                nc.sync.dma_start(
                    out=o_cnt[:, c0:c0 + cw], in_=ccopy[:, 0:cw])

    nc.compile()
    return nc


_NC_CACHE = {}


def _get_nc(je, sbw):
    key = (je, sbw)
    if key not in _NC_CACHE:
        _NC_CACHE[key] = build_bass_full(je, sbw)
    return _NC_CACHE[key]


def _prep(y, yh, status):
    """Host prep: fp16 rounding, bucketing, packing."""
    y16 = np.asarray(y, np.float32).astype(ml_dtypes.bfloat16)
    yh16 = np.asarray(yh, np.float32).astype(ml_dtypes.bfloat16)
    st = np.asarray(status)
    ev = np.nonzero(st == 1)[0]
    ns = len(ev)
    je = max(CH, int(math.ceil(ns / 128)) * 128)

    # rank-buckets of yh16, ties by index (stable)
    order = np.argsort(yh16.astype(np.float32), kind="stable")
    rank = np.empty(N, np.int64)
    rank[order] = np.arange(N)
    q = (rank * K) // N              # exactly N/K = 128 per bucket
    qj = q[ev]

    yj = np.full((1, je), PAD, ml_dtypes.bfloat16)
    yj[0, :ns] = y16[ev]

    members = order.reshape(K, P)    # bucket b = order[b*128:(b+1)*128]
    ev_by_b = [ev[qj == b] for b in range(K)]
    w = max(8, int(math.ceil(max((len(e) for e in ev_by_b), default=1) / 8)) * 8)
    sbw = (K // NCORES) * w          # 16 buckets per core

    return dict(y16=y16, yh16=yh16, ev=ev, ns=ns, je=je, q=q, qj=qj, yj=yj,
                members=members, ev_by_b=ev_by_b, w=w, sbw=sbw)


def make_in_maps(pp):
    y16, yh16 = pp["y16"], pp["yh16"]
    q, members, ev_by_b, w, sbw = (pp["q"], pp["members"], pp["ev_by_b"],
                                   pp["w"], pp["sbw"])
    in_maps = []
    for c in range(NCORES):
        i0 = c * TPC
        idx = np.arange(i0, i0 + TPC).reshape(NT, P)   # [t, p]
        y_is = y16.astype(np.float32)[idx].T.copy()    # [P, NT]
        u_w = np.zeros((P, NT, K), ml_dtypes.float8_e4m3)
        for t in range(NT):
            qi = q[idx[t]]
            val = -0.5 if t in SIGN_TILES else 1.0
            u_w[np.arange(P), t, qi] = val
        nb = K // NCORES
        sb_yj = np.full((P, nb, w), PAD, ml_dtypes.bfloat16)
        sb_yi = np.zeros((P, nb, w), ml_dtypes.bfloat16)
        sb_hj = np.full((P, nb, w), PAD, ml_dtypes.bfloat16)
        sb_hi = np.zeros((P, nb, w), ml_dtypes.bfloat16)
        for bi in range(nb):
            b = c * nb + bi
            mi = members[b]
            e = ev_by_b[b]
            sb_yi[:, bi, :] = y16[mi][:, None]
            sb_hi[:, bi, :] = yh16[mi][:, None]
            if len(e):
                sb_yj[:, bi, :len(e)] = y16[e][None, :]
                sb_hj[:, bi, :len(e)] = yh16[e][None, :]
        in_maps.append({
            "yj": pp["yj"],
            "y_is": np.ascontiguousarray(y_is),
            "u_w": np.ascontiguousarray(u_w.reshape(P, NT * K)),
            "sb_yj": np.ascontiguousarray(sb_yj.reshape(P, sbw)),
            "sb_yi": np.ascontiguousarray(sb_yi.reshape(P, sbw)),
            "sb_hj": np.ascontiguousarray(sb_hj.reshape(P, sbw)),
            "sb_hi": np.ascontiguousarray(sb_hi.reshape(P, sbw)),
        })
    return in_maps


def combine(results, pp):
    ns, je, q, qj, ev = pp["ns"], pp["je"], pp["q"], pp["qj"], pp["ev"]
    S1x = 0.0
    S1s = 0.0
    S2 = 0.0
    n_srows = len(SIGN_TILES) * P       # sign-tile i rows per core
    for c, r in enumerate(results):
        cnt = r["o_cnt"].astype(np.float64)[:, :ns]   # drop pad columns
        i0 = c * TPC
        idx = np.arange(i0, i0 + TPC).reshape(NT, P)
        nk_sign = np.zeros(K)
        for t in SIGN_TILES:
            nk_sign += np.bincount(q[idx[t]], minlength=K)
        # S2: sum of all cnt (01 tiles give A; sign tiles -g/2) + const:
        # per (i in sign, j real): A = (1-g)/2 -> sum A = ns*n_srows/2 - sum g/2
        S2 += cnt.sum() + ns * n_srows / 2.0
        # S1 cross: sum_{k>q_j} cnt[k,j] via cumsum + gather, + sign const
        csum = cnt.cumsum(axis=0)
        S1x += (csum[-1, :] - csum[qj, np.arange(ns)]).sum()
        suf_nk = np.concatenate([np.cumsum(nk_sign[::-1])[::-1][1:], [0.0]])
        S1x += suf_nk[qj].sum() / 2.0
        # diag fix: sign-tile diagonal gives g=0 (A should be 1 -> +1/2)
        in_core = ev[(ev >= i0) & (ev < i0 + TPC)]
        t_of = (in_core - i0) // P
        S2 += 0.5 * sum(1 for tt in t_of if tt in SIGN_TILES)
        S1s += float(r["o_sbacc"].astype(np.float64).sum())
    c_cnt = S1x + S1s - ns
    t_cnt = S2 - ns
    return np.asarray(np.float32(np.float32(c_cnt) / np.float32(t_cnt)))


def kernel(y, y_hat, status, _run_kwargs=None):
    pp = _prep(y, y_hat, status)
    nc = _get_nc(pp["je"], pp["sbw"])
    in_maps = make_in_maps(pp)
    kw = dict(_run_kwargs or {})
    res = bass_utils.run_bass_kernel_spmd(
        nc, in_maps, core_ids=list(range(NCORES)), **kw)
    out = combine(res.results, pp)
    if _run_kwargs is not None:
        return out, res
    return out


if __name__ == "__main__":
    rng = np.random.default_rng(0)
    y = rng.standard_normal(N).astype(np.float32)
    yh = rng.standard_normal(N).astype(np.float32)
    st = (rng.integers(0, 2, N)).astype(np.int32)
    print(kernel(y, yh, st))


# revision 3
# speedup vs baseline: 1.3430x; 1.3430x over previous
"""C-index kernel v3 for Trainium2, 8 NeuronCores — histogram scheme.

Math (bf16-rounded world; error vs fp32 reference ~1e-3 << 2e-2 gate)
----
y, yh rounded to bf16; both rank-bucketed into K=128 equal buckets
(exactly 128 samples each): p(.) for y, q(.) for yh.  Over the
rectangle (i in [N]) x (j in events E), with
    [y_i>=y_j]  = [p_i>p_j] + [p_i==p_j][y_i>=y_j]
    [yh_i>=yh_j] = [q_i>q_j] + [q_i==q_j][yh_i>=yh_j]
the product expands into four disjoint terms:
    S1 = T_bb + T_byh + T_yb + T_yy
    T_bb  = sum [p_i>p_j][q_i>q_j]                  (histogram product)
    T_byh = sum [q_i==q_j][p_i>p_j][yh_i>=yh_j]     (same-yh-bucket pass)
    T_yb  = sum [p_i==p_j][y_i>=y_j][q_i>q_j]       (same-y-bucket pass)
    T_yy  = sum [p_i==p_j][q_i==q_j][y_i>=y_j][yh_i>=yh_j]
    S2 = T2_b + T2_y;  T2_b = sum [p_i>p_j] (1-D histograms, host),
    T2_y = sum [p_i==p_j][y_i>=y_j] (same-y-bucket pass)
    c = S1 - ns, t = S2 - ns, out = fp32(c)/fp32(t)

Device (i sharded 8 ways = 2048 rows = 16 one-hot tiles; 16 buckets of
each kind per core):
  - Hi = sum_i onehot_p(i) x onehot_q(i): 8 fp8 DoubleRow matmuls into
    PSUM [128, 128]; T_bb = fused mult+accum against host-built
    W[r,s] = #events with p_j<r and q_j<s.
  - bucket passes: packed [128 members x W events] tiles, bf16
    tensor_tensor compares + fused scalar_tensor_tensor accumulates,
    all on DVE (GpSimd is banned: concurrent Pool+DVE ops slow each
    other ~25x on TRN2, measured).
Host: bf16 rounding, argsort bucketing, event histogram + suffix,
packing; final fp64 algebra over [128]-vector accumulators.
"""

import math
import os
import sys

import numpy as np
import ml_dtypes

for _p in ("/opt/trn_rl_repo", "/root/.axon_site", "/root/.axon_site/_ro/trn_rl_repo"):
    if os.path.isdir(_p) and _p not in sys.path:
        sys.path.append(_p)

import concourse.bacc as bacc
import concourse.mybir as mybir
from concourse import bass_utils
from concourse import tile

N = 16384
P = 128
NCORES = 8
TPC = N // NCORES          # 2048 i per core
NT = TPC // P              # 16 i-tiles per core
K = 128                    # buckets in each dim (N/K = 128 exactly)
NB = K // NCORES           # 16 buckets of each kind per core
BIGPAD = ml_dtypes.bfloat16(1e30)
IDXPAD = ml_dtypes.bfloat16(300.0)   # bucket-index pad (> any k)

FP32 = mybir.dt.float32
BF16 = mybir.dt.bfloat16
FP8 = mybir.dt.float8e4
Alu = mybir.AluOpType
DR = mybir.MatmulPerfMode.DoubleRow


def build_bass_full(wh, wy):
    """wh/wy: event width per yh-/y-bucket block (multiples of 8)."""
    sbh = NB * wh
    sby = NB * wy
    nc = bacc.Bacc(debug=False, num_devices=NCORES)

    uy = nc.dram_tensor("uy", [P, NT * K], FP8, kind="ExternalInput")
    uh = nc.dram_tensor("uh", [P, NT * K], FP8, kind="ExternalInput")
    wbb = nc.dram_tensor("wbb", [P, K], FP32, kind="ExternalInput")
    h_hj = nc.dram_tensor("h_hj", [P, sbh], BF16, kind="ExternalInput")
    h_hi = nc.dram_tensor("h_hi", [P, sbh], BF16, kind="ExternalInput")
    h_pj = nc.dram_tensor("h_pj", [P, sbh], BF16, kind="ExternalInput")
    h_pi = nc.dram_tensor("h_pi", [P, sbh], BF16, kind="ExternalInput")
    y_yj = nc.dram_tensor("y_yj", [P, sby], BF16, kind="ExternalInput")
    y_yi = nc.dram_tensor("y_yi", [P, sby], BF16, kind="ExternalInput")
    y_qj = nc.dram_tensor("y_qj", [P, sby], BF16, kind="ExternalInput")
    y_qi = nc.dram_tensor("y_qi", [P, sby], BF16, kind="ExternalInput")
    y_hj = nc.dram_tensor("y_hj", [P, sby], BF16, kind="ExternalInput")
    y_hi = nc.dram_tensor("y_hi", [P, sby], BF16, kind="ExternalInput")
    o_acc = nc.dram_tensor("o_acc", [P, 8], FP32, kind="ExternalOutput")

    with tile.TileContext(nc) as tc:
        with (
            tc.tile_pool(name="const", bufs=1) as cpool,
            tc.tile_pool(name="scrap", bufs=1) as spool,
            tc.tile_pool(name="psum", bufs=1, space="PSUM") as ppool,
        ):
            # ---- inputs ----
            t_uy = cpool.tile([P, NT, K], FP8, name="t_uy")
            nc.sync.dma_start(out=t_uy[:, :, :], in_=uy[:, :])
            t_uh = cpool.tile([P, NT, K], FP8, name="t_uh")
            nc.sync.dma_start(out=t_uh[:, :, :], in_=uh[:, :])
            t_wbb = cpool.tile([P, K], FP32, name="t_wbb")
            nc.sync.dma_start(out=t_wbb[:, :], in_=wbb[:, :])
            ins = {}
            for nm, dram, width in (
                ("h_hj", h_hj, sbh), ("h_hi", h_hi, sbh),
                ("h_pj", h_pj, sbh), ("h_pi", h_pi, sbh),
                ("y_yj", y_yj, sby), ("y_yi", y_yi, sby),
                ("y_qj", y_qj, sby), ("y_qi", y_qi, sby),
                ("y_hj", y_hj, sby), ("y_hi", y_hi, sby),
            ):
                t = cpool.tile([P, width], BF16, name=f"t_{nm}")
                nc.sync.dma_start(out=t[:, :], in_=dram[:, :])
                ins[nm] = t

            acc = cpool.tile([P, 8], FP32, name="acc")
            nc.vector.memset(acc[:, :], 0.0)

            # ---- Hi histogram + T_bb ----
            hi = ppool.tile([P, K], FP32, name="hi")
            for m in range(NT // 2):
                nc.tensor.matmul(
                    hi[:, :],
                    t_uy[:, 2 * m:2 * m + 2, :],
                    t_uh[:, 2 * m:2 * m + 2, :],
                    start=(m == 0), stop=(m == NT // 2 - 1),
                    perf_mode=DR)
            tbb_out = spool.tile([P, K], FP32, name="tbb_out")
            nc.vector.scalar_tensor_tensor(
                out=tbb_out[:, :], in0=hi[:, :], scalar=1.0,
                in1=t_wbb[:, :], op0=Alu.mult, op1=Alu.mult,
                accum_out=acc[:, 0:1])

            # ---- same-yh-bucket pass: T_byh ----
            cp = spool.tile([P, sbh], FP8, name="cp")
            nc.vector.tensor_tensor(
                out=cp[:, :], in0=ins["h_pj"][:, :], in1=ins["h_pi"][:, :],
                op=Alu.is_lt)
            c4h = spool.tile([P, sbh], FP8, name="c4h")
            nc.vector.tensor_tensor(
                out=c4h[:, :], in0=ins["h_hj"][:, :], in1=ins["h_hi"][:, :],
                op=Alu.is_le)
            m1 = spool.tile([P, sbh], FP8, name="m1")
            nc.vector.scalar_tensor_tensor(
                out=m1[:, :], in0=cp[:, :], scalar=1.0, in1=c4h[:, :],
                op0=Alu.mult, op1=Alu.mult, accum_out=acc[:, 1:2])

            # ---- same-y-bucket pass: T_yb, T_yy, T2_y ----
            c1 = spool.tile([P, sby], FP8, name="c1")
            nc.vector.tensor_tensor(
                out=c1[:, :], in0=ins["y_yj"][:, :], in1=ins["y_yi"][:, :],
                op=Alu.is_le)
            t2y = spool.tile([P, sby], FP8, name="t2y")
            nc.vector.tensor_scalar(
                out=t2y[:, :], in0=c1[:, :], scalar1=1.0, scalar2=0.0,
                op0=Alu.mult, op1=Alu.add, accum_out=acc[:, 4:5])
            cq = spool.tile([P, sby], FP8, name="cq")
            nc.vector.tensor_tensor(
                out=cq[:, :], in0=ins["y_qj"][:, :], in1=ins["y_qi"][:, :],
                op=Alu.is_lt)
            m2 = spool.tile([P, sby], FP8, name="m2")
            nc.vector.scalar_tensor_tensor(
                out=m2[:, :], in0=cq[:, :], scalar=1.0, in1=c1[:, :],
                op0=Alu.mult, op1=Alu.mult, accum_out=acc[:, 2:3])
            ceq = spool.tile([P, sby], FP8, name="ceq")
            nc.vector.tensor_tensor(
                out=ceq[:, :], in0=ins["y_qj"][:, :], in1=ins["y_qi"][:, :],
                op=Alu.is_equal)
            c4y = spool.tile([P, sby], FP8, name="c4y")
            nc.vector.tensor_tensor(
                out=c4y[:, :], in0=ins["y_hj"][:, :], in1=ins["y_hi"][:, :],
                op=Alu.is_le)
            m3 = spool.tile([P, sby], FP8, name="m3")
            nc.vector.tensor_tensor(
                out=m3[:, :], in0=ceq[:, :], in1=c4y[:, :], op=Alu.mult)
            m4 = spool.tile([P, sby], FP8, name="m4")
            nc.vector.scalar_tensor_tensor(
                out=m4[:, :], in0=m3[:, :], scalar=1.0, in1=c1[:, :],
                op0=Alu.mult, op1=Alu.mult, accum_out=acc[:, 3:4])

            nc.sync.dma_start(out=o_acc[:, :], in_=acc[:, :])

    nc.compile()
    return nc


_NC_CACHE = {}


def _get_nc(wh, wy):
    key = (wh, wy)
    if key not in _NC_CACHE:
        _NC_CACHE[key] = build_bass_full(wh, wy)
    return _NC_CACHE[key]


def _rank_buckets(v32):
    order = np.argsort(v32, kind="stable")
    r = np.empty(N, np.int64)
    r[order] = np.arange(N)
    return (r * K) // N, order


def _prep(y, yh, status):
    y16 = np.asarray(y, np.float32).astype(ml_dtypes.bfloat16)
    yh16 = np.asarray(yh, np.float32).astype(ml_dtypes.bfloat16)
    st = np.asarray(status)
    ev = np.nonzero(st == 1)[0]
    ns = len(ev)
    p, order_y = _rank_buckets(y16.astype(np.float32))
    q, order_h = _rank_buckets(yh16.astype(np.float32))
    pj, qj = p[ev], q[ev]

    # W[r,s] = #events with p_j < r and q_j < s (strict 2-D prefix)
    hj2 = np.zeros((K, K))
    np.add.at(hj2, (pj, qj), 1.0)
    pref = hj2.cumsum(0).cumsum(1)
    w_strict = np.zeros((K, K), np.float32)
    w_strict[1:, 1:] = pref[:-1, :-1]

    mem_y = order_y.reshape(K, P)
    mem_h = order_h.reshape(K, P)
    ev_by_p = [ev[pj == b] for b in range(K)]
    ev_by_q = [ev[qj == b] for b in range(K)]
    wy = max(8, int(math.ceil(max(len(e) for e in ev_by_p) / 8)) * 8)
    wh = max(8, int(math.ceil(max(len(e) for e in ev_by_q) / 8)) * 8)

    return dict(y16=y16, yh16=yh16, ev=ev, ns=ns, p=p, q=q, pj=pj, qj=qj,
                w_strict=w_strict, mem_y=mem_y, mem_h=mem_h,
                ev_by_p=ev_by_p, ev_by_q=ev_by_q, wy=wy, wh=wh)


def make_in_maps(pp):
    y16, yh16, p, q = pp["y16"], pp["yh16"], pp["p"], pp["q"]
    wy, wh = pp["wy"], pp["wh"]
    b16 = ml_dtypes.bfloat16
    p16 = p.astype(b16)
    q16 = q.astype(b16)
    in_maps = []
    for c in range(NCORES):
        i0 = c * TPC
        idx = np.arange(i0, i0 + TPC).reshape(NT, P)
        u_y = np.zeros((P, NT, K), ml_dtypes.float8_e4m3)
        u_h = np.zeros((P, NT, K), ml_dtypes.float8_e4m3)
        for t in range(NT):
            u_y[np.arange(P), t, p[idx[t]]] = 1.0
            u_h[np.arange(P), t, q[idx[t]]] = 1.0

        def pack(mem, evb, w, jvals, ivals, jpad):
            """[P, NB, w]: block b: col w = event w of bucket, row = member."""
            aj = np.full((P, NB, w), jpad, b16)
            ai = np.zeros((P, NB, w), b16)
            for bi in range(NB):
                b = c * NB + bi
                e = evb[b]
                ai[:, bi, :] = ivals[mem[b]][:, None]
                if len(e):
                    aj[:, bi, :len(e)] = jvals[e][None, :]
            return np.ascontiguousarray(aj.reshape(P, NB * w)), \
                np.ascontiguousarray(ai.reshape(P, NB * w))

        h_hj, h_hi = pack(pp["mem_h"], pp["ev_by_q"], wh, yh16, yh16, BIGPAD)
        h_pj, h_pi = pack(pp["mem_h"], pp["ev_by_q"], wh, p16, p16, IDXPAD)
        y_yj, y_yi = pack(pp["mem_y"], pp["ev_by_p"], wy, y16, y16, BIGPAD)
        y_qj, y_qi = pack(pp["mem_y"], pp["ev_by_p"], wy, q16, q16, IDXPAD)
        y_hj, y_hi = pack(pp["mem_y"], pp["ev_by_p"], wy, yh16, yh16, BIGPAD)

        # wbb row r aligns with PSUM partition r (p-bucket)
        in_maps.append({
            "uy": np.ascontiguousarray(u_y.reshape(P, NT * K)),
            "uh": np.ascontiguousarray(u_h.reshape(P, NT * K)),
            "wbb": np.ascontiguousarray(pp["w_strict"]),
            "h_hj": h_hj, "h_hi": h_hi, "h_pj": h_pj, "h_pi": h_pi,
            "y_yj": y_yj, "y_yi": y_yi, "y_qj": y_qj, "y_qi": y_qi,
            "y_hj": y_hj, "y_hi": y_hi,
        })
    return in_maps


def combine(results, pp):
    ns, p, pj = pp["ns"], pp["p"], pp["pj"]
    S1 = 0.0
    S2 = 0.0
    for r in results:
        a = r["o_acc"].astype(np.float64)
        S1 += a[:, 0].sum() + a[:, 1].sum() + a[:, 2].sum() + a[:, 3].sum()
        S2 += a[:, 4].sum()
    # host: T2_b = sum_j #{i: p_i > p_j}
    h1 = np.bincount(p, minlength=K)
    suf1 = np.concatenate([np.cumsum(h1[::-1])[::-1][1:], [0]])
    S2 += float(suf1[pj].sum())
    c_cnt = S1 - ns
    t_cnt = S2 - ns
    return np.asarray(np.float32(np.float32(c_cnt) / np.float32(t_cnt)))


def kernel(y, y_hat, status, _run_kwargs=None):
    pp = _prep(y, y_hat, status)
    nc = _get_nc(pp["wh"], pp["wy"])
    in_maps = make_in_maps(pp)
    kw = dict(_run_kwargs or {})
    res = bass_utils.run_bass_kernel_spmd(
        nc, in_maps, core_ids=list(range(NCORES)), **kw)
    out = combine(res.results, pp)
    if _run_kwargs is not None:
        return out, res
    return out


if __name__ == "__main__":
    rng = np.random.default_rng(0)
    y = rng.standard_normal(N).astype(np.float32)
    yh = rng.standard_normal(N).astype(np.float32)
    st = (rng.integers(0, 2, N)).astype(np.int32)
    print(kernel(y, yh, st))


# revision 4
# speedup vs baseline: 1.3546x; 1.0086x over previous
"""C-index kernel v3 for Trainium2, 8 NeuronCores — histogram scheme.

Math (bf16-rounded world; error vs fp32 reference ~1e-3 << 2e-2 gate)
----
y, yh rounded to bf16; both rank-bucketed into K=128 equal buckets
(exactly 128 samples each): p(.) for y, q(.) for yh.  Over the
rectangle (i in [N]) x (j in events E), with
    [y_i>=y_j]  = [p_i>p_j] + [p_i==p_j][y_i>=y_j]
    [yh_i>=yh_j] = [q_i>q_j] + [q_i==q_j][yh_i>=yh_j]
the product expands into four disjoint terms:
    S1 = T_bb + T_byh + T_yb + T_yy
    T_bb  = sum [p_i>p_j][q_i>q_j]                  (histogram product)
    T_byh = sum [q_i==q_j][p_i>p_j][yh_i>=yh_j]     (same-yh-bucket pass)
    T_yb  = sum [p_i==p_j][y_i>=y_j][q_i>q_j]       (same-y-bucket pass)
    T_yy  = sum [p_i==p_j][q_i==q_j][y_i>=y_j][yh_i>=yh_j]
    S2 = T2_b + T2_y;  T2_b = sum [p_i>p_j] (1-D histograms, host),
    T2_y = sum [p_i==p_j][y_i>=y_j] (same-y-bucket pass)
    c = S1 - ns, t = S2 - ns, out = fp32(c)/fp32(t)

Device (i sharded 8 ways = 2048 rows = 16 one-hot tiles; 16 buckets of
each kind per core):
  - Hi = sum_i onehot_p(i) x onehot_q(i): 8 fp8 DoubleRow matmuls into
    PSUM [128, 128]; T_bb = fused mult+accum against host-built
    W[r,s] = #events with p_j<r and q_j<s.
  - bucket passes: packed [128 members x W events] tiles, bf16
    tensor_tensor compares + fused scalar_tensor_tensor accumulates,
    all on DVE (GpSimd is banned: concurrent Pool+DVE ops slow each
    other ~25x on TRN2, measured).
Host: bf16 rounding, argsort bucketing, event histogram + suffix,
packing; final fp64 algebra over [128]-vector accumulators.
"""

import math
import os
import sys

import numpy as np
import ml_dtypes

for _p in ("/opt/trn_rl_repo", "/root/.axon_site", "/root/.axon_site/_ro/trn_rl_repo"):
    if os.path.isdir(_p) and _p not in sys.path:
        sys.path.append(_p)

import concourse.bacc as bacc
import concourse.mybir as mybir
from concourse import bass_utils
from concourse import tile

N = 16384
P = 128
NCORES = 8
TPC = N // NCORES          # 2048 i per core
NT = TPC // P              # 16 i-tiles per core
K = 128                    # buckets in each dim (N/K = 128 exactly)
NB = K // NCORES           # 16 buckets of each kind per core
BIGPAD = ml_dtypes.bfloat16(1e30)
IDXPAD = ml_dtypes.bfloat16(300.0)   # bucket-index pad (> any k)

FP32 = mybir.dt.float32
BF16 = mybir.dt.bfloat16
FP8 = mybir.dt.float8e4
Alu = mybir.AluOpType
DR = mybir.MatmulPerfMode.DoubleRow


def build_bass_full(sbh, sby):
    """sbh/sby: total packed widths of the yh-/y-bucket passes."""
    nc = bacc.Bacc(debug=False, num_devices=NCORES)

    uu = nc.dram_tensor("uu", [P, 2 * NT * K], FP8, kind="ExternalInput")
    wbb = nc.dram_tensor("wbb", [P, K], FP32, kind="ExternalInput")
    harr = nc.dram_tensor("harr", [P, 4 * sbh], BF16, kind="ExternalInput")
    yarr = nc.dram_tensor("yarr", [P, 6 * sby], BF16, kind="ExternalInput")
    o_acc = nc.dram_tensor("o_acc", [P, 5], FP32, kind="ExternalOutput")

    with tile.TileContext(nc) as tc:
        with (
            tc.tile_pool(name="const", bufs=1) as cpool,
            tc.tile_pool(name="scrap", bufs=1) as spool,
            tc.tile_pool(name="psum", bufs=1, space="PSUM") as ppool,
        ):
            # ---- inputs: per-array sub-DMAs in consumption order ----
            t_h = cpool.tile([P, 4, sbh], BF16, name="t_h")
            t_y = cpool.tile([P, 6, sby], BF16, name="t_y")
            qengs = [nc.sync, nc.scalar]
            for xi in range(4):
                qengs[xi % 2].dma_start(
                    out=t_h[:, xi, :], in_=harr[:, xi * sbh:(xi + 1) * sbh])
            for xi in range(6):
                qengs[xi % 2].dma_start(
                    out=t_y[:, xi, :], in_=yarr[:, xi * sby:(xi + 1) * sby])
            ins = {
                "h_pj": t_h[:, 0, :], "h_pi": t_h[:, 1, :],
                "h_hj": t_h[:, 2, :], "h_hi": t_h[:, 3, :],
                "y_yj": t_y[:, 0, :], "y_yi": t_y[:, 1, :],
                "y_qj": t_y[:, 2, :], "y_qi": t_y[:, 3, :],
                "y_hj": t_y[:, 4, :], "y_hi": t_y[:, 5, :],
            }
            t_uu = cpool.tile([P, 2, NT, K], FP8, name="t_uu")
            nc.sync.dma_start(out=t_uu[:, :, :, :], in_=uu[:, :])
            t_wbb = cpool.tile([P, K], FP32, name="t_wbb")
            nc.scalar.dma_start(out=t_wbb[:, :], in_=wbb[:, :])

            accs = [cpool.tile([P, 1], FP32, name=f"acc{x}")
                    for x in range(5)]

            # ---- Hi histogram + T_bb ----
            hi = ppool.tile([P, K], FP32, name="hi")
            for m in range(NT // 2):
                nc.tensor.matmul(
                    hi[:, :],
                    t_uu[:, 0, 2 * m:2 * m + 2, :],
                    t_uu[:, 1, 2 * m:2 * m + 2, :],
                    start=(m == 0), stop=(m == NT // 2 - 1),
                    perf_mode=DR)
            # ---- same-yh-bucket pass: T_byh ----
            cp = spool.tile([P, sbh], BF16, name="cp")
            nc.vector.tensor_tensor(
                out=cp[:, :], in0=ins["h_pj"], in1=ins["h_pi"],
                op=Alu.is_lt)
            c4h = spool.tile([P, sbh], BF16, name="c4h")
            nc.vector.tensor_tensor(
                out=c4h[:, :], in0=ins["h_hj"], in1=ins["h_hi"],
                op=Alu.is_le)
            m1 = spool.tile([P, sbh], BF16, name="m1")
            nc.vector.scalar_tensor_tensor(
                out=m1[:, :], in0=cp[:, :], scalar=1.0, in1=c4h[:, :],
                op0=Alu.mult, op1=Alu.mult, accum_out=accs[1][:, :])

            # ---- same-y-bucket pass: T_yb, T_yy, T2_y ----
            c1 = spool.tile([P, sby], BF16, name="c1")
            nc.vector.tensor_tensor(
                out=c1[:, :], in0=ins["y_yj"], in1=ins["y_yi"],
                op=Alu.is_le)
            t2y = spool.tile([P, sby], BF16, name="t2y")
            nc.vector.tensor_scalar(
                out=t2y[:, :], in0=c1[:, :], scalar1=1.0, scalar2=0.0,
                op0=Alu.mult, op1=Alu.add, accum_out=accs[4][:, :])
            cq = spool.tile([P, sby], BF16, name="cq")
            nc.vector.tensor_tensor(
                out=cq[:, :], in0=ins["y_qj"], in1=ins["y_qi"],
                op=Alu.is_lt)
            m2 = spool.tile([P, sby], BF16, name="m2")
            nc.vector.scalar_tensor_tensor(
                out=m2[:, :], in0=cq[:, :], scalar=1.0, in1=c1[:, :],
                op0=Alu.mult, op1=Alu.mult, accum_out=accs[2][:, :])
            ceq = spool.tile([P, sby], BF16, name="ceq")
            nc.vector.tensor_tensor(
                out=ceq[:, :], in0=ins["y_qj"], in1=ins["y_qi"],
                op=Alu.is_equal)
            c4y = spool.tile([P, sby], BF16, name="c4y")
            nc.vector.tensor_tensor(
                out=c4y[:, :], in0=ins["y_hj"], in1=ins["y_hi"],
                op=Alu.is_le)
            m3 = spool.tile([P, sby], BF16, name="m3")
            nc.vector.tensor_tensor(
                out=m3[:, :], in0=ceq[:, :], in1=c4y[:, :], op=Alu.mult)
            m4 = spool.tile([P, sby], BF16, name="m4")
            nc.vector.scalar_tensor_tensor(
                out=m4[:, :], in0=m3[:, :], scalar=1.0, in1=c1[:, :],
                op0=Alu.mult, op1=Alu.mult, accum_out=accs[3][:, :])

            tbb_out = spool.tile([P, K], FP32, name="tbb_out")
            nc.vector.scalar_tensor_tensor(
                out=tbb_out[:, :], in0=hi[:, :], scalar=1.0,
                in1=t_wbb[:, :], op0=Alu.mult, op1=Alu.mult,
                accum_out=accs[0][:, :])

            accg = cpool.tile([P, 5], FP32, name="accg")
            for x in range(5):
                nc.vector.tensor_copy(out=accg[:, x:x + 1], in_=accs[x][:, :])
            nc.sync.dma_start(out=o_acc[:, 0:5], in_=accg[:, :])

    nc.compile()
    return nc


_NC_CACHE = {}


def _get_nc(sbh, sby):
    key = (sbh, sby)
    if key not in _NC_CACHE:
        _NC_CACHE[key] = build_bass_full(sbh, sby)
    return _NC_CACHE[key]


def _rank_buckets(v32):
    order = np.argsort(v32, kind="stable")
    r = np.empty(N, np.int64)
    r[order] = np.arange(N)
    return (r * K) // N, order


def _prep(y, yh, status):
    y16 = np.asarray(y, np.float32).astype(ml_dtypes.bfloat16)
    yh16 = np.asarray(yh, np.float32).astype(ml_dtypes.bfloat16)
    st = np.asarray(status)
    ev = np.nonzero(st == 1)[0]
    ns = len(ev)
    p, order_y = _rank_buckets(y16.astype(np.float32))
    q, order_h = _rank_buckets(yh16.astype(np.float32))
    pj, qj = p[ev], q[ev]

    # W[r,s] = #events with p_j < r and q_j < s (strict 2-D prefix)
    hj2 = np.zeros((K, K))
    np.add.at(hj2, (pj, qj), 1.0)
    pref = hj2.cumsum(0).cumsum(1)
    w_strict = np.zeros((K, K), np.float32)
    w_strict[1:, 1:] = pref[:-1, :-1]

    mem_y = order_y.reshape(K, P)
    mem_h = order_h.reshape(K, P)
    ev_by_p = [ev[pj == b] for b in range(K)]
    ev_by_q = [ev[qj == b] for b in range(K)]

    def layout(evb):
        offs = []
        tot = 0
        for c in range(NCORES):
            o = [0]
            for bi in range(NB):
                w = int(math.ceil(max(1, len(evb[c * NB + bi])) / 8)) * 8
                o.append(o[-1] + w)
            offs.append(o)
            tot = max(tot, o[-1])
        return offs, tot

    offs_y, sby = layout(ev_by_p)
    offs_h, sbh = layout(ev_by_q)

    return dict(y16=y16, yh16=yh16, ev=ev, ns=ns, p=p, q=q, pj=pj, qj=qj,
                w_strict=w_strict, mem_y=mem_y, mem_h=mem_h,
                ev_by_p=ev_by_p, ev_by_q=ev_by_q,
                offs_y=offs_y, sby=sby, offs_h=offs_h, sbh=sbh)


def make_in_maps(pp):
    y16, yh16, p, q = pp["y16"], pp["yh16"], pp["p"], pp["q"]
    b16 = ml_dtypes.bfloat16
    p16 = p.astype(b16)
    q16 = q.astype(b16)
    in_maps = []
    for c in range(NCORES):
        i0 = c * TPC
        idx = np.arange(i0, i0 + TPC).reshape(NT, P)
        u_y = np.zeros((P, NT, K), ml_dtypes.float8_e4m3)
        u_h = np.zeros((P, NT, K), ml_dtypes.float8_e4m3)
        for t in range(NT):
            u_y[np.arange(P), t, p[idx[t]]] = 1.0
            u_h[np.arange(P), t, q[idx[t]]] = 1.0

        def pack(mem, evb, offs, tot, jvals, ivals, jpad):
            """[P, tot]: concatenated per-bucket blocks (variable width)."""
            aj = np.full((P, tot), jpad, b16)
            ai = np.zeros((P, tot), b16)
            for bi in range(NB):
                b = c * NB + bi
                e = evb[b]
                o0, o1 = offs[bi], offs[bi + 1]
                ai[:, o0:o1] = ivals[mem[b]][:, None]
                aj[:, o0:o0 + len(e)] = jvals[e][None, :]
            return np.ascontiguousarray(aj), np.ascontiguousarray(ai)

        offs_h, sbh = pp["offs_h"][c], pp["sbh"]
        offs_y, sby = pp["offs_y"][c], pp["sby"]
        h_hj, h_hi = pack(pp["mem_h"], pp["ev_by_q"], offs_h, sbh,
                          yh16, yh16, BIGPAD)
        h_pj, h_pi = pack(pp["mem_h"], pp["ev_by_q"], offs_h, sbh,
                          p16, p16, IDXPAD)
        y_yj, y_yi = pack(pp["mem_y"], pp["ev_by_p"], offs_y, sby,
                          y16, y16, BIGPAD)
        y_qj, y_qi = pack(pp["mem_y"], pp["ev_by_p"], offs_y, sby,
                          q16, q16, IDXPAD)
        y_hj, y_hi = pack(pp["mem_y"], pp["ev_by_p"], offs_y, sby,
                          yh16, yh16, BIGPAD)

        # wbb row r aligns with PSUM partition r (p-bucket)
        in_maps.append({
            "uu": np.ascontiguousarray(
                np.stack([u_y, u_h], axis=1).reshape(P, 2 * NT * K)),
            "wbb": np.ascontiguousarray(pp["w_strict"]),
            "harr": np.ascontiguousarray(
                np.concatenate([h_pj, h_pi, h_hj, h_hi], axis=1)),
            "yarr": np.ascontiguousarray(
                np.concatenate([y_yj, y_yi, y_qj, y_qi, y_hj, y_hi], axis=1)),
        })
    return in_maps


def combine(results, pp):
    ns, p, pj = pp["ns"], pp["p"], pp["pj"]
    S1 = 0.0
    S2 = 0.0
    for r in results:
        a = r["o_acc"].astype(np.float64)
        S1 += a[:, 0].sum() + a[:, 1].sum() + a[:, 2].sum() + a[:, 3].sum()
        S2 += a[:, 4].sum()
    # host: T2_b = sum_j #{i: p_i > p_j}
    h1 = np.bincount(p, minlength=K)
    suf1 = np.concatenate([np.cumsum(h1[::-1])[::-1][1:], [0]])
    S2 += float(suf1[pj].sum())
    c_cnt = S1 - ns
    t_cnt = S2 - ns
    return np.asarray(np.float32(np.float32(c_cnt) / np.float32(t_cnt)))


def kernel(y, y_hat, status, _run_kwargs=None):
    pp = _prep(y, y_hat, status)
    nc = _get_nc(pp["sbh"], pp["sby"])
    in_maps = make_in_maps(pp)
    kw = dict(_run_kwargs or {})
    res = bass_utils.run_bass_kernel_spmd(
        nc, in_maps, core_ids=list(range(NCORES)), **kw)
    out = combine(res.results, pp)
    if _run_kwargs is not None:
        return out, res
    return out


if __name__ == "__main__":
    rng = np.random.default_rng(0)
    y = rng.standard_normal(N).astype(np.float32)
    yh = rng.standard_normal(N).astype(np.float32)
    st = (rng.integers(0, 2, N)).astype(np.int32)
    print(kernel(y, yh, st))


# revision 5
# speedup vs baseline: 1.5528x; 1.1464x over previous
"""C-index kernel v3 for Trainium2, 8 NeuronCores — histogram scheme.

Math (bf16-rounded world; error vs fp32 reference ~1e-3 << 2e-2 gate)
----
y, yh rounded to bf16; both rank-bucketed into K=128 equal buckets
(exactly 128 samples each): p(.) for y, q(.) for yh.  Over the
rectangle (i in [N]) x (j in events E), with
    [y_i>=y_j]  = [p_i>p_j] + [p_i==p_j][y_i>=y_j]
    [yh_i>=yh_j] = [q_i>q_j] + [q_i==q_j][yh_i>=yh_j]
the product expands into four disjoint terms:
    S1 = T_bb + T_byh + T_yb + T_yy
    T_bb  = sum [p_i>p_j][q_i>q_j]                  (histogram product)
    T_byh = sum [q_i==q_j][p_i>p_j][yh_i>=yh_j]     (same-yh-bucket pass)
    T_yb  = sum [p_i==p_j][y_i>=y_j][q_i>q_j]       (same-y-bucket pass)
    T_yy  = sum [p_i==p_j][q_i==q_j][y_i>=y_j][yh_i>=yh_j]
    S2 = T2_b + T2_y;  T2_b = sum [p_i>p_j] (1-D histograms, host),
    T2_y = sum [p_i==p_j][y_i>=y_j] (same-y-bucket pass)
    c = S1 - ns, t = S2 - ns, out = fp32(c)/fp32(t)

Device (i sharded 8 ways = 2048 rows = 16 one-hot tiles; 16 buckets of
each kind per core):
  - Hi = sum_i onehot_p(i) x onehot_q(i): 8 fp8 DoubleRow matmuls into
    PSUM [128, 128]; T_bb = fused mult+accum against host-built
    W[r,s] = #events with p_j<r and q_j<s.
  - bucket passes: packed [128 members x W events] tiles, bf16
    tensor_tensor compares + fused scalar_tensor_tensor accumulates,
    all on DVE (GpSimd is banned: concurrent Pool+DVE ops slow each
    other ~25x on TRN2, measured).
Host: bf16 rounding, argsort bucketing, event histogram + suffix,
packing; final fp64 algebra over [128]-vector accumulators.
"""

import math
import os
import sys

import numpy as np
import ml_dtypes

for _p in ("/opt/trn_rl_repo", "/root/.axon_site", "/root/.axon_site/_ro/trn_rl_repo"):
    if os.path.isdir(_p) and _p not in sys.path:
        sys.path.append(_p)

import concourse.bacc as bacc
import concourse.mybir as mybir
from concourse import bass_utils
from concourse import tile

N = 16384
P = 128
NCORES = 8
TPC = N // NCORES          # 2048 i per core
NT = TPC // P              # 16 i-tiles per core
K = 128                    # buckets in each dim (N/K = 128 exactly)
NB = K // NCORES           # 16 buckets of each kind per core
BIGPAD = ml_dtypes.bfloat16(1e30)
IDXPAD = ml_dtypes.bfloat16(300.0)   # bucket-index pad (> any k)

FP32 = mybir.dt.float32
BF16 = mybir.dt.bfloat16
FP8 = mybir.dt.float8e4
Alu = mybir.AluOpType
ActF = mybir.ActivationFunctionType
DR = mybir.MatmulPerfMode.DoubleRow


def build_bass_full(sbh, sby):
    """sbh/sby: total packed widths of the yh-/y-bucket passes."""
    nc = bacc.Bacc(debug=False, num_devices=NCORES)

    uu = nc.dram_tensor("uu", [P, 2 * NT * K], FP8, kind="ExternalInput")
    wbb = nc.dram_tensor("wbb", [P, K], FP32, kind="ExternalInput")
    harr = nc.dram_tensor("harr", [P, 4 * sbh], BF16, kind="ExternalInput")
    yarr = nc.dram_tensor("yarr", [P, 4 * sby], BF16, kind="ExternalInput")
    o_acc = nc.dram_tensor("o_acc", [P, 4], FP32, kind="ExternalOutput")

    with tile.TileContext(nc) as tc:
        with (
            tc.tile_pool(name="const", bufs=1) as cpool,
            tc.tile_pool(name="scrap", bufs=1) as spool,
            tc.tile_pool(name="psum", bufs=1, space="PSUM") as ppool,
        ):
            # ---- inputs: per-array sub-DMAs in consumption order ----
            t_h = cpool.tile([P, 4, sbh], BF16, name="t_h")
            t_y = cpool.tile([P, 4, sby], BF16, name="t_y")
            qengs = [nc.sync, nc.scalar]
            for xi in range(4):
                qengs[xi % 2].dma_start(
                    out=t_h[:, xi, :], in_=harr[:, xi * sbh:(xi + 1) * sbh])
            for xi in range(4):
                qengs[xi % 2].dma_start(
                    out=t_y[:, xi, :], in_=yarr[:, xi * sby:(xi + 1) * sby])
            ins = {
                "h_pj": t_h[:, 0, :], "h_pi": t_h[:, 1, :],
                "h_hj": t_h[:, 2, :], "h_hi": t_h[:, 3, :],
                "y_yj": t_y[:, 0, :], "y_yi": t_y[:, 1, :],
                "y_hj": t_y[:, 2, :], "y_hi": t_y[:, 3, :],
            }
            t_uu = cpool.tile([P, 2, NT, K], FP8, name="t_uu")
            nc.sync.dma_start(out=t_uu[:, :, :, :], in_=uu[:, :])
            t_wbb = cpool.tile([P, K], FP32, name="t_wbb")
            nc.scalar.dma_start(out=t_wbb[:, :], in_=wbb[:, :])

            accs = [cpool.tile([P, 1], FP32, name=f"acc{x}")
                    for x in range(4)]

            # ---- Hi histogram + T_bb ----
            hi = ppool.tile([P, K], FP32, name="hi")
            for m in range(NT // 2):
                nc.tensor.matmul(
                    hi[:, :],
                    t_uu[:, 0, 2 * m:2 * m + 2, :],
                    t_uu[:, 1, 2 * m:2 * m + 2, :],
                    start=(m == 0), stop=(m == NT // 2 - 1),
                    perf_mode=DR)
            # ---- same-yh-bucket pass: T_byh ----
            cp = spool.tile([P, sbh], BF16, name="cp")
            nc.vector.tensor_tensor(
                out=cp[:, :], in0=ins["h_pj"], in1=ins["h_pi"],
                op=Alu.is_lt)
            c4h = spool.tile([P, sbh], BF16, name="c4h")
            nc.vector.tensor_tensor(
                out=c4h[:, :], in0=ins["h_hj"], in1=ins["h_hi"],
                op=Alu.is_le)
            m1 = spool.tile([P, sbh], BF16, name="m1")
            nc.vector.scalar_tensor_tensor(
                out=m1[:, :], in0=cp[:, :], scalar=1.0, in1=c4h[:, :],
                op0=Alu.mult, op1=Alu.mult, accum_out=accs[1][:, :])

            # ---- same-y-bucket pass ----
            # within a y-bucket, [q>]+[q=][yh>=] == [yh>=], so the whole
            # same-y-bucket contribution is sum [y>=][yh>=]
            c1 = spool.tile([P, sby], BF16, name="c1")
            nc.vector.tensor_tensor(
                out=c1[:, :], in0=ins["y_yj"], in1=ins["y_yi"],
                op=Alu.is_le)
            t2y = spool.tile([P, sby], BF16, name="t2y")
            nc.scalar.activation(
                out=t2y[:, :], in_=c1[:, :], func=ActF.Copy,
                bias=0.0, scale=1.0, accum_out=accs[3][:, :])
            c4y = spool.tile([P, sby], BF16, name="c4y")
            nc.vector.tensor_tensor(
                out=c4y[:, :], in0=ins["y_hj"], in1=ins["y_hi"],
                op=Alu.is_le)
            m4 = spool.tile([P, sby], BF16, name="m4")
            nc.vector.scalar_tensor_tensor(
                out=m4[:, :], in0=c4y[:, :], scalar=1.0, in1=c1[:, :],
                op0=Alu.mult, op1=Alu.mult, accum_out=accs[2][:, :])

            tbb_out = spool.tile([P, K], FP32, name="tbb_out")
            nc.vector.scalar_tensor_tensor(
                out=tbb_out[:, :], in0=hi[:, :], scalar=1.0,
                in1=t_wbb[:, :], op0=Alu.mult, op1=Alu.mult,
                accum_out=accs[0][:, :])

            for x in range(4):
                nc.sync.dma_start(out=o_acc[:, x:x + 1], in_=accs[x][:, :])

    nc.compile()
    return nc


_NC_CACHE = {}


def _get_nc(sbh, sby):
    key = (sbh, sby)
    if key not in _NC_CACHE:
        _NC_CACHE[key] = build_bass_full(sbh, sby)
    return _NC_CACHE[key]


def _rank_buckets(v32):
    order = np.argsort(v32, kind="stable")
    r = np.empty(N, np.int64)
    r[order] = np.arange(N)
    return (r * K) // N, order


def _prep(y, yh, status):
    y16 = np.asarray(y, np.float32).astype(ml_dtypes.bfloat16)
    yh16 = np.asarray(yh, np.float32).astype(ml_dtypes.bfloat16)
    st = np.asarray(status)
    ev = np.nonzero(st == 1)[0]
    ns = len(ev)
    p, order_y = _rank_buckets(y16.astype(np.float32))
    q, order_h = _rank_buckets(yh16.astype(np.float32))
    pj, qj = p[ev], q[ev]

    # W[r,s] = #events with p_j < r and q_j < s (strict 2-D prefix)
    hj2 = np.zeros((K, K))
    np.add.at(hj2, (pj, qj), 1.0)
    pref = hj2.cumsum(0).cumsum(1)
    w_strict = np.zeros((K, K), np.float32)
    w_strict[1:, 1:] = pref[:-1, :-1]

    mem_y = order_y.reshape(K, P)
    mem_h = order_h.reshape(K, P)
    ev_by_p = [ev[pj == b] for b in range(K)]
    ev_by_q = [ev[qj == b] for b in range(K)]

    def layout(evb):
        offs = []
        tot = 0
        for c in range(NCORES):
            o = [0]
            for bi in range(NB):
                w = int(math.ceil(max(1, len(evb[c * NB + bi])) / 8)) * 8
                o.append(o[-1] + w)
            offs.append(o)
            tot = max(tot, o[-1])
        return offs, tot

    offs_y, sby = layout(ev_by_p)
    offs_h, sbh = layout(ev_by_q)

    return dict(y16=y16, yh16=yh16, ev=ev, ns=ns, p=p, q=q, pj=pj, qj=qj,
                w_strict=w_strict, mem_y=mem_y, mem_h=mem_h,
                ev_by_p=ev_by_p, ev_by_q=ev_by_q,
                offs_y=offs_y, sby=sby, offs_h=offs_h, sbh=sbh)


def make_in_maps(pp):
    y16, yh16, p, q = pp["y16"], pp["yh16"], pp["p"], pp["q"]
    b16 = ml_dtypes.bfloat16
    p16 = p.astype(b16)
    q16 = q.astype(b16)
    in_maps = []
    for c in range(NCORES):
        i0 = c * TPC
        idx = np.arange(i0, i0 + TPC).reshape(NT, P)
        u_y = np.zeros((P, NT, K), ml_dtypes.float8_e4m3)
        u_h = np.zeros((P, NT, K), ml_dtypes.float8_e4m3)
        for t in range(NT):
            u_y[np.arange(P), t, p[idx[t]]] = 1.0
            u_h[np.arange(P), t, q[idx[t]]] = 1.0

        def pack(mem, evb, offs, tot, jvals, ivals, jpad):
            """[P, tot]: concatenated per-bucket blocks (variable width)."""
            aj = np.full((P, tot), jpad, b16)
            ai = np.zeros((P, tot), b16)
            for bi in range(NB):
                b = c * NB + bi
                e = evb[b]
                o0, o1 = offs[bi], offs[bi + 1]
                ai[:, o0:o1] = ivals[mem[b]][:, None]
                aj[:, o0:o0 + len(e)] = jvals[e][None, :]
            return np.ascontiguousarray(aj), np.ascontiguousarray(ai)

        offs_h, sbh = pp["offs_h"][c], pp["sbh"]
        offs_y, sby = pp["offs_y"][c], pp["sby"]
        h_hj, h_hi = pack(pp["mem_h"], pp["ev_by_q"], offs_h, sbh,
                          yh16, yh16, BIGPAD)
        h_pj, h_pi = pack(pp["mem_h"], pp["ev_by_q"], offs_h, sbh,
                          p16, p16, IDXPAD)
        y_yj, y_yi = pack(pp["mem_y"], pp["ev_by_p"], offs_y, sby,
                          y16, y16, BIGPAD)
        y_hj, y_hi = pack(pp["mem_y"], pp["ev_by_p"], offs_y, sby,
                          yh16, yh16, BIGPAD)

        # wbb row r aligns with PSUM partition r (p-bucket)
        in_maps.append({
            "uu": np.ascontiguousarray(
                np.stack([u_y, u_h], axis=1).reshape(P, 2 * NT * K)),
            "wbb": np.ascontiguousarray(pp["w_strict"]),
            "harr": np.ascontiguousarray(
                np.concatenate([h_pj, h_pi, h_hj, h_hi], axis=1)),
            "yarr": np.ascontiguousarray(
                np.concatenate([y_yj, y_yi, y_hj, y_hi], axis=1)),
        })
    return in_maps


def combine(results, pp):
    ns, p, pj = pp["ns"], pp["p"], pp["pj"]
    S1 = 0.0
    S2 = 0.0
    for r in results:
        a = r["o_acc"].astype(np.float64)
        S1 += a[:, 0].sum() + a[:, 1].sum() + a[:, 2].sum()
        S2 += a[:, 3].sum()
    # host: T2_b = sum_j #{i: p_i > p_j}
    h1 = np.bincount(p, minlength=K)
    suf1 = np.concatenate([np.cumsum(h1[::-1])[::-1][1:], [0]])
    S2 += float(suf1[pj].sum())
    c_cnt = S1 - ns
    t_cnt = S2 - ns
    return np.asarray(np.float32(np.float32(c_cnt) / np.float32(t_cnt)))


def kernel(y, y_hat, status, _run_kwargs=None):
    pp = _prep(y, y_hat, status)
    nc = _get_nc(pp["sbh"], pp["sby"])
    in_maps = make_in_maps(pp)
    kw = dict(_run_kwargs or {})
    res = bass_utils.run_bass_kernel_spmd(
        nc, in_maps, core_ids=list(range(NCORES)), **kw)
    out = combine(res.results, pp)
    if _run_kwargs is not None:
        return out, res
    return out


if __name__ == "__main__":
    rng = np.random.default_rng(0)
    y = rng.standard_normal(N).astype(np.float32)
    yh = rng.standard_normal(N).astype(np.float32)
    st = (rng.integers(0, 2, N)).astype(np.int32)
    print(kernel(y, yh, st))
